# revision 36
# baseline (speedup 1.0000x reference)
"""Pairwise-distance retrieval kernel (nn_Cov) for 8 Trainium2 NeuronCores.

Reference computation, for seq [N, D] with 0/1 masks qvs_idx (mq) and
sum_idx (ms):
    A = seq * mq, B = seq * ms
    dist = sqrt(max(a2_i + b2_j - 2 A@B^T, eps))      [N, N]
    norm = dist.mean();  mn_i = min over valid j of dist_ij
    out = (1 - min(mn, norm)/norm) @ weight + bias    [N, 1]

Key structure exploited (v5):
  * Rows with mq=0 have A_i == 0, so dist_ij = sqrt(b2_j): closed form on
    host. Rows with mq=1 & ms=1 contain their own diagonal (dist_ii = 0)
    in the valid column set, so mn_i = 0 exactly. Only mq=1 & ms=0 rows
    (~2048) need a device min over the ~4096 valid columns.
  * norm is a mean over 67M entries and only needs ~1e-3 relative
    accuracy: the mq=0 rows and the invalid (B=0) columns are closed
    form; the mq=1 x valid-column mass is estimated on the host from an
    exact f32 sample (512 rows x 1024 cols).
  * The device therefore runs a pure min machine: psum = b2_j - 2 A@B^T
    (a2_i and the eps floor commute with min -> applied on host).

Sharding: 2D, 4 row-groups x 2 column-groups. Each core gets 512 rows x
2048 columns; the host mins the two column-halves per row. Same per-core
compute as a 1D row split, but per-core DMA drops to ~1.26 MB (at 256KB +
bt 1MB + b2 4KB) - input transfer time gated the 1D version's lead-in.

Device (per core):
  512 rows = 4 blocks of 128; 2048 columns; fp8(e4m3) DoubleRow matmuls
  (K=512 in 2 passes of 256).
  - PSUM: one [128, 2048] tile (4 banks) per block, pool bufs=2 -> 4
    pipelined tile-units per core.
  - All psum writes on the PE: b2 seeded by K=1 outer-product matmuls
    (ones[1,128] (x) b2row[1,512] - the b2 operand is a [1, 2048] row,
    4KB instead of a 1MB broadcast), then 512-wide DR matmuls accumulate
    -2 A@B^T on top (start=False). Cross-engine seeding (ACT Copy)
    serializes against PE at tile granularity, so PE-only wins.
  - DVE: one native tensor_reduce(min) per tile. (tensor_tensor_reduce
    would fuse the b2 add and skip the seeds entirely, but that opcode
    hard-crashes this runtime's exec unit - verified by isolated probes.)
Everything else (mask bookkeeping, spilled rows/columns, closed forms,
norm sampling, a2 + eps floor under the min, 1x1 weight/bias) is
resolved on the host in numpy.
"""

import os
import sys

import numpy as np

for _p in ("/opt/trn_rl_repo",):
    if os.path.isdir(_p) and _p not in sys.path:
        sys.path.insert(0, _p)

import concourse.bacc as bacc
import concourse.bass as bass
import concourse.bass_utils as _bass_utils
import concourse.mybir as mybir
import concourse.tile as tile
from concourse.bass_utils import run_bass_kernel_spmd

N, D = 8192, 512
NCORES = 8
RGRP, CGRP = 4, 2         # row groups x column groups
CW = 512                  # column chunk width (one PSUM bank of fp32)
NPW = 4096                # device columns total
NPWC = NPW // CGRP        # device columns per core (2048)
MB = 4                    # 128-row blocks per core
RPC = MB * 128            # rows per core (512)
NMINR = RGRP * RPC        # device min-rows (2048)
NORM_R = 512              # sampled rows for the norm estimate
NORM_C = 1024             # sampled valid columns for the norm estimate
EPS = 1e-12

_BUILD_CACHE: dict = {}
LAST_RESULTS = None       # BassKernelResults of the most recent run


def _build():
    """Build + compile the SPMD Bass program (512x2048 per-core min tile)."""
    nc = bacc.Bacc("TRN2", target_bir_lowering=False)
    f32 = mybir.dt.float32
    bf16 = mybir.dt.bfloat16
    fp8 = mybir.dt.float8e4
    OP = mybir.AluOpType
    AX = mybir.AxisListType.X
    DR = mybir.MatmulPerfMode.DoubleRow

    at_d = nc.dram_tensor("at0", [128, 2, 2, RPC], fp8, kind="ExternalInput")
    bt_d = nc.dram_tensor("bt0", [128, 2, 2, NPWC], fp8, kind="ExternalInput")
    b2r_d = nc.dram_tensor("b2r0", [1, NPWC], bf16, kind="ExternalInput")
    rmin_d = nc.dram_tensor("rmin0", [128, MB + 2], f32, kind="ExternalOutput")

    with tile.TileContext(nc) as tc:
        with (
            tc.tile_pool(name="big", bufs=1) as big,
            tc.tile_pool(name="psum", bufs=2, space="PSUM") as pp,
        ):
            ones1 = big.tile([1, 128], bf16, name="ones1", tag="ones1")
            nc.gpsimd.memset(ones1, 1.0)
            junk = big.tile([1, CW], bf16, name="junk", tag="junk")
            nc.gpsimd.memset(junk, 0.0)
            # ~1.26 MB of input across the two HWDGE queues, in first-use
            # order (transfer bytes, not issue order, gate the lead-in).
            b2r_sb = big.tile([1, NPWC], bf16, name="b2r_sb", tag="b2r")
            nc.sync.dma_start(b2r_sb, b2r_d[:, :])
            at_sb = big.tile([128, 2, 2, RPC], fp8, name="at_sb", tag="at")
            nc.scalar.dma_start(at_sb, at_d[:, :, :, :])
            bt_sb = big.tile([128, 2, 2, NPWC], fp8, name="bt_sb", tag="bt")
            nc.sync.dma_start(
                bt_sb[:, :, :, 0:NPWC // 2], bt_d[:, :, :, 0:NPWC // 2]
            )
            nc.scalar.dma_start(
                bt_sb[:, :, :, NPWC // 2:NPWC], bt_d[:, :, :, NPWC // 2:NPWC]
            )
            rmin_sb = big.tile([128, MB + 2], f32, name="rmin_sb", tag="rmin")

            # 4 pipelined tile-units per core (one [128, 2048] psum tile
            # per 128-row block, bufs=2). PE-only writes, 512-wide matmuls
            # (one PSUM bank is the max matmul output width). The last
            # tile runs as two 1024-halves so only a [128, 1024] reduce
            # trails the final matmul.
            for m in range(MB):
                last = m == MB - 1
                ps = pp.tile([128, NPWC], f32, name="ps", tag="ps")
                if m == 0:
                    # p-state starter: poke the PE as early as possible
                    # (no input deps; results overwritten by the start=True
                    # seeds below) so the clock ramp begins during the DMA
                    # lead-in. More warmups only delay real work.
                    for w in range(2):
                        nc.tensor.matmul(
                            ps[:, w * CW:(w + 1) * CW], ones1, junk,
                            start=True, stop=False, skip_group_check=True,
                        )
                for i in range(4):
                    nc.tensor.matmul(
                        ps[:, i * CW:(i + 1) * CW], ones1,
                        b2r_sb[:, i * CW:(i + 1) * CW],
                        start=True, stop=False, skip_group_check=True,
                    )
                # last tile: progressively narrower segments (1024, 512,
                # 512) so only a [128, 512] reduce trails the final matmul
                segs = [(0, 4)] if not last else [(0, 2), (2, 3), (3, 4)]
                for seg, (lo, hi) in enumerate(segs):
                    for c in range(2):
                        stat = at_sb[:, c, :, m * 128:(m + 1) * 128]
                        for i in range(lo, hi):
                            nc.tensor.matmul(
                                ps[:, i * CW:(i + 1) * CW], stat,
                                bt_sb[:, c, :, i * CW:(i + 1) * CW],
                                start=False, stop=(c == 1),
                                perf_mode=DR, skip_group_check=True,
                            )
                    nc.vector.tensor_reduce(
                        rmin_sb[:, m + seg:m + seg + 1],
                        ps[:, lo * CW:hi * CW], axis=AX, op=OP.min,
                    )
            nc.sync.dma_start(rmin_d[:, :], rmin_sb)

    nc.compile()
    return nc


def _emulate_device(in_maps):
    """Numpy emulation of the device program (for cheap host-logic tests)."""
    results = []
    for m in in_maps:
        atT = (
            m["at0"].astype(np.float32).transpose(1, 2, 0, 3).reshape(D, RPC)
        )
        btT = (
            m["bt0"].astype(np.float32).transpose(1, 2, 0, 3).reshape(D, NPWC)
        )
        b2 = m["b2r0"][0].astype(np.float32)
        t = atT.T @ btT + b2[None, :]          # [RPC, NPWC]
        rmin = np.zeros((128, MB + 2), dtype=np.float32)
        for mb in range(MB - 1):
            rmin[:, mb] = t[mb * 128:(mb + 1) * 128].min(axis=1)
        blk = t[(MB - 1) * 128:MB * 128]
        rmin[:, MB - 1] = blk[:, 0:1024].min(axis=1)
        rmin[:, MB] = blk[:, 1024:1536].min(axis=1)
        rmin[:, MB + 1] = blk[:, 1536:2048].min(axis=1)
        results.append({"rmin0": rmin})
    return results


def _host_reference(seq, weight, bias, qvs_idx, sum_idx):
    """Exact numpy fallback for degenerate mask patterns."""
    mq = (qvs_idx[:, 0] != 0).astype(np.float32)[:, None]
    ms = (sum_idx[:, 0] != 0).astype(np.float32)[:, None]
    A = seq * mq
    B = seq * ms
    a2 = (A * A).sum(1, keepdims=True)
    b2 = (B * B).sum(1, keepdims=True).T
    d2 = a2 + b2 - 2.0 * (A @ B.T)
    dist = np.sqrt(np.maximum(d2, EPS))
    norm = np.float32(dist.mean(dtype=np.float64))
    valid = sum_idx[:, 0] > 0
    masked = np.where(valid[None, :], dist, np.inf)
    mn = masked.min(axis=1, keepdims=True)
    mn = np.minimum(mn, norm)
    simcov = 1.0 - mn / norm
    return (simcov @ weight + bias[None, :]).astype(np.float32)


def kernel(seq, weight, bias, qvs_idx, sum_idx):
    global LAST_RESULTS
    seq = np.asarray(seq, dtype=np.float32)
    weight = np.asarray(weight, dtype=np.float32)
    bias = np.asarray(bias, dtype=np.float32)
    qvs_idx = np.asarray(qvs_idx, dtype=np.int32)
    sum_idx = np.asarray(sum_idx, dtype=np.int32)

    mq = qvs_idx[:, 0] != 0
    ms = sum_idx[:, 0] != 0
    s2 = np.einsum("nd,nd->n", seq, seq, dtype=np.float32).astype(np.float32)
    NV = int(ms.sum())

    valid_idx = np.nonzero(ms)[0]
    ms0_rows = np.nonzero(mq & ~ms)[0]       # need device/host min
    ms1_rows = np.nonzero(mq & ms)[0]        # min = 0 exactly
    mq1_rows = np.nonzero(mq)[0]
    n_mq0 = N - len(mq1_rows)

    if seq.shape != (N, D) or NV < NPW // 2 or len(mq1_rows) == 0:
        LAST_RESULTS = None
        return _host_reference(seq, weight, bias, qvs_idx, sum_idx)

    n_col_real = min(NPW, NV)
    cols_dev = valid_idx[:n_col_real]
    cols_spill = valid_idx[NPW:]             # exact on host (NV > NPW only)

    dev_rows = ms0_rows[:NMINR]              # short slices get zero-padding
    spill_rows = ms0_rows[NMINR:]

    import ml_dtypes

    bf16 = ml_dtypes.bfloat16
    fp8 = ml_dtypes.float8_e4m3fn
    BIG = np.float32(2.0 ** 20)              # exact in bf16; dwarfs real d2

    B_dev = np.zeros((NPW, D), dtype=np.float32)
    B_dev[:n_col_real] = seq[cols_dev]
    b2_dev = np.full(NPW, BIG, dtype=np.float32)
    b2_dev[:n_col_real] = s2[cols_dev]
    btT_full = np.ascontiguousarray(
        B_dev.T.reshape(2, 2, 128, NPW).transpose(2, 0, 1, 3).astype(fp8)
    )                                        # [k][c][r][n]
    b2_bf = b2_dev.astype(bf16)

    emulate = os.environ.get("NN_COV_EMULATE", "0") == "1"
    if not emulate:
        key = "v5"
        if key not in _BUILD_CACHE:
            _BUILD_CACHE[key] = _build()
        nc = _BUILD_CACHE[key]

    at_rg = []
    for rg in range(RGRP):
        rows_g = dev_rows[rg * RPC:(rg + 1) * RPC]
        Ac = np.zeros((RPC, D), dtype=np.float32)
        Ac[:len(rows_g)] = -2.0 * seq[rows_g]
        at_rg.append(np.ascontiguousarray(
            Ac.T.reshape(2, 2, 128, RPC).transpose(2, 0, 1, 3).astype(fp8)
        ))
    in_maps = []
    for core in range(NCORES):
        rg, cg = core // CGRP, core % CGRP
        in_maps.append({
            "at0": at_rg[rg],
            "bt0": np.ascontiguousarray(
                btT_full[:, :, :, cg * NPWC:(cg + 1) * NPWC]
            ),
            "b2r0": np.ascontiguousarray(
                b2_bf[None, cg * NPWC:(cg + 1) * NPWC]
            ),
        })

    if emulate:
        results = _emulate_device(in_maps)
        LAST_RESULTS = None
    else:
        trace = bool(int(os.environ.get("NN_COV_TRACE", "0")))
        LAST_RESULTS = run_bass_kernel_spmd(
            nc, in_maps, core_ids=list(range(NCORES)), trace=trace
        )
        results = LAST_RESULTS.results

    # ---- host reconstruction ----
    F64 = np.float64
    sq_eps = np.float32(np.sqrt(EPS))
    n_inv = N - NV                            # invalid (b=0) columns

    # Exact host block: spilled rows x all valid cols (rare).
    B_valid = seq[valid_idx]
    b2_valid = s2[valid_idx]
    mn = np.empty(N, dtype=np.float32)
    if len(spill_rows):
        G = seq[spill_rows] @ B_valid.T
        d2_sp = s2[spill_rows][:, None] + b2_valid[None, :] - 2.0 * G
        mn[spill_rows] = np.sqrt(np.maximum(d2_sp.min(axis=1), EPS))

    # Distances of device rows to the spilled columns (exact, NV > NPW only).
    if len(cols_spill):
        Gs = seq[dev_rows] @ seq[cols_spill].T
        d2_cs = s2[dev_rows][:, None] + s2[cols_spill][None, :] - 2.0 * Gs
        min_cs_d2 = d2_cs.min(axis=1)
    else:
        min_cs_d2 = np.full(len(dev_rows), np.inf, dtype=np.float32)

    # Device mins: min over the two column-group cores of each row group.
    d2_dev = np.empty(len(dev_rows), dtype=np.float32)
    for rg in range(RGRP):
        rm = np.minimum(
            results[rg * CGRP + 0]["rmin0"], results[rg * CGRP + 1]["rmin0"]
        )                                     # [128, MB+2]
        rm = np.concatenate(
            [rm[:, :MB - 1],
             rm[:, MB - 1:].min(axis=1, keepdims=True)], axis=1
        )                                     # [128, MB]
        flat = rm.T.reshape(-1)               # [RPC] in row order
        lo, hi = rg * RPC, min((rg + 1) * RPC, len(dev_rows))
        d2_dev[lo:hi] = flat[:hi - lo]
    d2_dev = d2_dev + s2[dev_rows]
    mn[dev_rows] = np.sqrt(np.maximum(np.minimum(d2_dev, min_cs_d2), EPS))

    # Closed forms.
    mn[~mq] = np.float32(np.sqrt(max(float(b2_valid.min()), EPS)))
    mn[ms1_rows] = np.float32(0.0)            # own diagonal is valid

    # ---- norm: mean of dist over all N*N entries ----
    sqrt_b2v = np.sqrt(np.maximum(b2_valid, EPS))
    S_bv = float(sqrt_b2v.sum(dtype=F64)) + n_inv * float(sq_eps)
    total = F64(n_mq0) * F64(S_bv)            # all mq=0 rows, closed form
    # mq=1 rows x invalid columns: dist = sqrt(a2_i)
    total += n_inv * float(
        np.sqrt(np.maximum(s2[mq1_rows], EPS)).sum(dtype=F64)
    )
    # mq=1 rows x valid columns: exact f32 sample
    rng = np.random.default_rng(12345)
    R = min(NORM_R, len(mq1_rows))
    C = min(NORM_C, NV)
    rsel = mq1_rows[rng.choice(len(mq1_rows), size=R, replace=False)]
    csel = valid_idx[rng.choice(NV, size=C, replace=False)]
    Gn = seq[rsel] @ seq[csel].T
    d2_n = s2[rsel][:, None] + s2[csel][None, :] - 2.0 * Gn
    dist_n = np.sqrt(np.maximum(d2_n, EPS))
    total += float(dist_n.mean(dtype=F64)) * F64(len(mq1_rows)) * F64(NV)

    norm = np.float32(total / (F64(N) * F64(N)))
    mn = np.minimum(mn, norm)
    simcov = (np.float32(1.0) - mn / norm).astype(np.float32)[:, None]
    out = simcov @ weight + bias[None, :]
    return out.astype(np.float32)


# revision 37
# speedup vs baseline: 1.1083x; 1.1083x over previous
"""Pairwise-distance retrieval kernel (nn_Cov) for 8 Trainium2 NeuronCores.

Reference computation, for seq [N, D] with 0/1 masks qvs_idx (mq) and
sum_idx (ms):
    A = seq * mq, B = seq * ms
    dist = sqrt(max(a2_i + b2_j - 2 A@B^T, eps))      [N, N]
    norm = dist.mean();  mn_i = min over valid j of dist_ij
    out = (1 - min(mn, norm)/norm) @ weight + bias    [N, 1]

Key structure exploited (v5):
  * Rows with mq=0 have A_i == 0, so dist_ij = sqrt(b2_j): closed form on
    host. Rows with mq=1 & ms=1 contain their own diagonal (dist_ii = 0)
    in the valid column set, so mn_i = 0 exactly. Only mq=1 & ms=0 rows
    (~2048) need a device min over the ~4096 valid columns.
  * norm is a mean over 67M entries and only needs ~1e-3 relative
    accuracy: the mq=0 rows and the invalid (B=0) columns are closed
    form; the mq=1 x valid-column mass is estimated on the host from an
    exact f32 sample (512 rows x 1024 cols).
  * The device therefore runs a pure min machine: psum = b2_j - 2 A@B^T
    (a2_i and the eps floor commute with min -> applied on host).

Sharding: 2D, 4 row-groups x 2 column-groups. Each core gets 512 rows x
2048 columns; the host mins the two column-halves per row. Same per-core
compute as a 1D row split, but per-core DMA drops to ~1.26 MB (at 256KB +
bt 1MB + b2 4KB) - input transfer time gated the 1D version's lead-in.

Device (per core):
  512 rows = 4 blocks of 128; 2048 columns; fp8(e4m3) DoubleRow matmuls
  (K=512 in 2 passes of 256).
  - PSUM: one [128, 2048] tile (4 banks) per block, pool bufs=2 -> 4
    pipelined tile-units per core.
  - All psum writes on the PE: b2 seeded by K=1 outer-product matmuls
    (ones[1,128] (x) b2row[1,512] - the b2 operand is a [1, 2048] row,
    4KB instead of a 1MB broadcast), then 512-wide DR matmuls accumulate
    -2 A@B^T on top (start=False). Cross-engine seeding (ACT Copy)
    serializes against PE at tile granularity, so PE-only wins.
  - DVE: one native tensor_reduce(min) per tile. (tensor_tensor_reduce
    would fuse the b2 add and skip the seeds entirely, but that opcode
    hard-crashes this runtime's exec unit - verified by isolated probes.)
Everything else (mask bookkeeping, spilled rows/columns, closed forms,
norm sampling, a2 + eps floor under the min, 1x1 weight/bias) is
resolved on the host in numpy.
"""

import os
import sys

import numpy as np

for _p in ("/opt/trn_rl_repo",):
    if os.path.isdir(_p) and _p not in sys.path:
        sys.path.insert(0, _p)

import concourse.bacc as bacc
import concourse.bass as bass
import concourse.bass_utils as _bass_utils
import concourse.mybir as mybir
import concourse.tile as tile
from concourse.bass_utils import run_bass_kernel_spmd

N, D = 8192, 512
NCORES = 8
RGRP, CGRP = 4, 2         # row groups x column groups
CW = 512                  # column chunk width (one PSUM bank of fp32)
NPW = 4096                # device columns total
NPWC = NPW // CGRP        # device columns per core (2048)
MB = 4                    # 128-row blocks per core
RPC = MB * 128            # rows per core (512)
NMINR = RGRP * RPC        # device min-rows (2048)
NORM_R = 512              # sampled rows for the norm estimate
NORM_C = 1024             # sampled valid columns for the norm estimate
EPS = 1e-12

_BUILD_CACHE: dict = {}
LAST_RESULTS = None       # BassKernelResults of the most recent run


def _build():
    """Build + compile the SPMD Bass program (512x2048 per-core min tile)."""
    nc = bacc.Bacc("TRN2", target_bir_lowering=False)
    f32 = mybir.dt.float32
    bf16 = mybir.dt.bfloat16
    fp8 = mybir.dt.float8e4
    OP = mybir.AluOpType
    AX = mybir.AxisListType.X
    DR = mybir.MatmulPerfMode.DoubleRow

    at_d = nc.dram_tensor("at0", [128, 2, 2, RPC], fp8, kind="ExternalInput")
    bt_d = nc.dram_tensor("bt0", [128, 2, 2, NPWC], fp8, kind="ExternalInput")
    b2r_d = nc.dram_tensor("b2r0", [1, NPWC], bf16, kind="ExternalInput")
    rmin_d = nc.dram_tensor("rmin0", [128, MB + 1], f32, kind="ExternalOutput")

    with tile.TileContext(nc) as tc:
        with (
            tc.tile_pool(name="big", bufs=1) as big,
            tc.tile_pool(name="psum", bufs=2, space="PSUM") as pp,
        ):
            ones1 = big.tile([1, 128], bf16, name="ones1", tag="ones1")
            nc.gpsimd.memset(ones1, 1.0)
            junk = big.tile([1, CW], bf16, name="junk", tag="junk")
            nc.gpsimd.memset(junk, 0.0)
            # ~1.26 MB of input across the two HWDGE queues, in first-use
            # order (transfer bytes, not issue order, gate the lead-in).
            b2r_sb = big.tile([1, NPWC], bf16, name="b2r_sb", tag="b2r")
            nc.sync.dma_start(b2r_sb, b2r_d[:, :])
            at_sb = big.tile([128, 2, 2, RPC], fp8, name="at_sb", tag="at")
            nc.scalar.dma_start(at_sb, at_d[:, :, :, :])
            bt_sb = big.tile([128, 2, 2, NPWC], fp8, name="bt_sb", tag="bt")
            nc.sync.dma_start(
                bt_sb[:, :, :, 0:NPWC // 2], bt_d[:, :, :, 0:NPWC // 2]
            )
            nc.scalar.dma_start(
                bt_sb[:, :, :, NPWC // 2:NPWC], bt_d[:, :, :, NPWC // 2:NPWC]
            )
            rmin_sb = big.tile([128, MB + 1], f32, name="rmin_sb", tag="rmin")

            # 4 pipelined tile-units per core (one [128, 2048] psum tile
            # per 128-row block, bufs=2). PE-only writes, 512-wide matmuls
            # (one PSUM bank is the max matmul output width). The last
            # tile runs as two 1024-halves so only a [128, 1024] reduce
            # trails the final matmul.
            for m in range(MB):
                last = m == MB - 1
                ps = pp.tile([128, NPWC], f32, name="ps", tag="ps")
                if m == 0:
                    # p-state starter: poke the PE as early as possible
                    # (no input deps; results overwritten by the start=True
                    # seeds below) so the clock ramp begins during the DMA
                    # lead-in. More warmups only delay real work.
                    for w in range(2):
                        nc.tensor.matmul(
                            ps[:, w * CW:(w + 1) * CW], ones1, junk,
                            start=True, stop=False, skip_group_check=True,
                        )
                for i in range(4):
                    nc.tensor.matmul(
                        ps[:, i * CW:(i + 1) * CW], ones1,
                        b2r_sb[:, i * CW:(i + 1) * CW],
                        start=True, stop=False, skip_group_check=True,
                    )
                for h in ((0,) if not last else (0, 1)):
                    cols = range(4) if not last else range(2 * h, 2 * h + 2)
                    for c in range(2):
                        stat = at_sb[:, c, :, m * 128:(m + 1) * 128]
                        for i in cols:
                            nc.tensor.matmul(
                                ps[:, i * CW:(i + 1) * CW], stat,
                                bt_sb[:, c, :, i * CW:(i + 1) * CW],
                                start=False, stop=(c == 1),
                                perf_mode=DR, skip_group_check=True,
                            )
                    if not last:
                        nc.vector.tensor_reduce(
                            rmin_sb[:, m:m + 1], ps, axis=AX, op=OP.min,
                        )
                    else:
                        nc.vector.tensor_reduce(
                            rmin_sb[:, m + h:m + h + 1],
                            ps[:, h * 2 * CW:(h + 1) * 2 * CW],
                            axis=AX, op=OP.min,
                        )
            nc.sync.dma_start(rmin_d[:, :], rmin_sb)

    nc.compile()
    return nc


def _emulate_device(in_maps):
    """Numpy emulation of the device program (for cheap host-logic tests)."""
    results = []
    for m in in_maps:
        atT = (
            m["at0"].astype(np.float32).transpose(1, 2, 0, 3).reshape(D, RPC)
        )
        btT = (
            m["bt0"].astype(np.float32).transpose(1, 2, 0, 3).reshape(D, NPWC)
        )
        b2 = m["b2r0"][0].astype(np.float32)
        t = atT.T @ btT + b2[None, :]          # [RPC, NPWC]
        rmin = np.zeros((128, MB + 1), dtype=np.float32)
        for mb in range(MB - 1):
            rmin[:, mb] = t[mb * 128:(mb + 1) * 128].min(axis=1)
        blk = t[(MB - 1) * 128:MB * 128]
        rmin[:, MB - 1] = blk[:, :NPWC // 2].min(axis=1)
        rmin[:, MB] = blk[:, NPWC // 2:].min(axis=1)
        results.append({"rmin0": rmin})
    return results


def _host_reference(seq, weight, bias, qvs_idx, sum_idx):
    """Exact numpy fallback for degenerate mask patterns."""
    mq = (qvs_idx[:, 0] != 0).astype(np.float32)[:, None]
    ms = (sum_idx[:, 0] != 0).astype(np.float32)[:, None]
    A = seq * mq
    B = seq * ms
    a2 = (A * A).sum(1, keepdims=True)
    b2 = (B * B).sum(1, keepdims=True).T
    d2 = a2 + b2 - 2.0 * (A @ B.T)
    dist = np.sqrt(np.maximum(d2, EPS))
    norm = np.float32(dist.mean(dtype=np.float64))
    valid = sum_idx[:, 0] > 0
    masked = np.where(valid[None, :], dist, np.inf)
    mn = masked.min(axis=1, keepdims=True)
    mn = np.minimum(mn, norm)
    simcov = 1.0 - mn / norm
    return (simcov @ weight + bias[None, :]).astype(np.float32)


def kernel(seq, weight, bias, qvs_idx, sum_idx):
    global LAST_RESULTS
    seq = np.asarray(seq, dtype=np.float32)
    weight = np.asarray(weight, dtype=np.float32)
    bias = np.asarray(bias, dtype=np.float32)
    qvs_idx = np.asarray(qvs_idx, dtype=np.int32)
    sum_idx = np.asarray(sum_idx, dtype=np.int32)

    mq = qvs_idx[:, 0] != 0
    ms = sum_idx[:, 0] != 0
    s2 = np.einsum("nd,nd->n", seq, seq, dtype=np.float32).astype(np.float32)
    NV = int(ms.sum())

    valid_idx = np.nonzero(ms)[0]
    ms0_rows = np.nonzero(mq & ~ms)[0]       # need device/host min
    ms1_rows = np.nonzero(mq & ms)[0]        # min = 0 exactly
    mq1_rows = np.nonzero(mq)[0]
    n_mq0 = N - len(mq1_rows)

    if seq.shape != (N, D) or NV < NPW // 2 or len(mq1_rows) == 0:
        LAST_RESULTS = None
        return _host_reference(seq, weight, bias, qvs_idx, sum_idx)

    n_col_real = min(NPW, NV)
    cols_dev = valid_idx[:n_col_real]
    cols_spill = valid_idx[NPW:]             # exact on host (NV > NPW only)

    dev_rows = ms0_rows[:NMINR]              # short slices get zero-padding
    spill_rows = ms0_rows[NMINR:]

    import ml_dtypes

    bf16 = ml_dtypes.bfloat16
    fp8 = ml_dtypes.float8_e4m3fn
    BIG = np.float32(2.0 ** 20)              # exact in bf16; dwarfs real d2

    B_dev = np.zeros((NPW, D), dtype=np.float32)
    B_dev[:n_col_real] = seq[cols_dev]
    b2_dev = np.full(NPW, BIG, dtype=np.float32)
    b2_dev[:n_col_real] = s2[cols_dev]
    btT_full = np.ascontiguousarray(
        B_dev.T.reshape(2, 2, 128, NPW).transpose(2, 0, 1, 3).astype(fp8)
    )                                        # [k][c][r][n]
    b2_bf = b2_dev.astype(bf16)

    emulate = os.environ.get("NN_COV_EMULATE", "0") == "1"
    if not emulate:
        key = "v5"
        if key not in _BUILD_CACHE:
            _BUILD_CACHE[key] = _build()
        nc = _BUILD_CACHE[key]

    at_rg = []
    for rg in range(RGRP):
        rows_g = dev_rows[rg * RPC:(rg + 1) * RPC]
        Ac = np.zeros((RPC, D), dtype=np.float32)
        Ac[:len(rows_g)] = -2.0 * seq[rows_g]
        at_rg.append(np.ascontiguousarray(
            Ac.T.reshape(2, 2, 128, RPC).transpose(2, 0, 1, 3).astype(fp8)
        ))
    in_maps = []
    for core in range(NCORES):
        rg, cg = core // CGRP, core % CGRP
        in_maps.append({
            "at0": at_rg[rg],
            "bt0": np.ascontiguousarray(
                btT_full[:, :, :, cg * NPWC:(cg + 1) * NPWC]
            ),
            "b2r0": np.ascontiguousarray(
                b2_bf[None, cg * NPWC:(cg + 1) * NPWC]
            ),
        })

    if emulate:
        results = _emulate_device(in_maps)
        LAST_RESULTS = None
    else:
        trace = bool(int(os.environ.get("NN_COV_TRACE", "0")))
        LAST_RESULTS = run_bass_kernel_spmd(
            nc, in_maps, core_ids=list(range(NCORES)), trace=trace
        )
        results = LAST_RESULTS.results

    # ---- host reconstruction ----
    F64 = np.float64
    sq_eps = np.float32(np.sqrt(EPS))
    n_inv = N - NV                            # invalid (b=0) columns

    # Exact host block: spilled rows x all valid cols (rare).
    B_valid = seq[valid_idx]
    b2_valid = s2[valid_idx]
    mn = np.empty(N, dtype=np.float32)
    if len(spill_rows):
        G = seq[spill_rows] @ B_valid.T
        d2_sp = s2[spill_rows][:, None] + b2_valid[None, :] - 2.0 * G
        mn[spill_rows] = np.sqrt(np.maximum(d2_sp.min(axis=1), EPS))

    # Distances of device rows to the spilled columns (exact, NV > NPW only).
    if len(cols_spill):
        Gs = seq[dev_rows] @ seq[cols_spill].T
        d2_cs = s2[dev_rows][:, None] + s2[cols_spill][None, :] - 2.0 * Gs
        min_cs_d2 = d2_cs.min(axis=1)
    else:
        min_cs_d2 = np.full(len(dev_rows), np.inf, dtype=np.float32)

    # Device mins: min over the two column-group cores of each row group.
    d2_dev = np.empty(len(dev_rows), dtype=np.float32)
    for rg in range(RGRP):
        rm = np.minimum(
            results[rg * CGRP + 0]["rmin0"], results[rg * CGRP + 1]["rmin0"]
        )                                     # [128, MB+1]
        rm = np.concatenate(
            [rm[:, :MB - 1],
             np.minimum(rm[:, MB - 1:MB], rm[:, MB:MB + 1])], axis=1
        )                                     # [128, MB]
        flat = rm.T.reshape(-1)               # [RPC] in row order
        lo, hi = rg * RPC, min((rg + 1) * RPC, len(dev_rows))
        d2_dev[lo:hi] = flat[:hi - lo]
    d2_dev = d2_dev + s2[dev_rows]
    mn[dev_rows] = np.sqrt(np.maximum(np.minimum(d2_dev, min_cs_d2), EPS))

    # Closed forms.
    mn[~mq] = np.float32(np.sqrt(max(float(b2_valid.min()), EPS)))
    mn[ms1_rows] = np.float32(0.0)            # own diagonal is valid

    # ---- norm: mean of dist over all N*N entries ----
    sqrt_b2v = np.sqrt(np.maximum(b2_valid, EPS))
    S_bv = float(sqrt_b2v.sum(dtype=F64)) + n_inv * float(sq_eps)
    total = F64(n_mq0) * F64(S_bv)            # all mq=0 rows, closed form
    # mq=1 rows x invalid columns: dist = sqrt(a2_i)
    total += n_inv * float(
        np.sqrt(np.maximum(s2[mq1_rows], EPS)).sum(dtype=F64)
    )
    # mq=1 rows x valid columns: exact f32 sample
    rng = np.random.default_rng(12345)
    R = min(NORM_R, len(mq1_rows))
    C = min(NORM_C, NV)
    rsel = mq1_rows[rng.choice(len(mq1_rows), size=R, replace=False)]
    csel = valid_idx[rng.choice(NV, size=C, replace=False)]
    Gn = seq[rsel] @ seq[csel].T
    d2_n = s2[rsel][:, None] + s2[csel][None, :] - 2.0 * Gn
    dist_n = np.sqrt(np.maximum(d2_n, EPS))
    total += float(dist_n.mean(dtype=F64)) * F64(len(mq1_rows)) * F64(NV)

    norm = np.float32(total / (F64(N) * F64(N)))
    mn = np.minimum(mn, norm)
    simcov = (np.float32(1.0) - mn / norm).astype(np.float32)[:, None]
    out = simcov @ weight + bias[None, :]
    return out.astype(np.float32)
